# revision 17
# baseline (speedup 1.0000x reference)
"""CTC batch loss (Keras convention, blank = C-1) on 8 Trainium2 NeuronCores.

Strategy (pure data parallel, 128 examples per core = 128 SBUF partitions):
  * Prob-domain scaled forward DP (mathematically identical to the reference's
    log-space DP, including the exact log(p + 1e-7) epsilon, which is folded
    into the gather as E = onehot + eps so gathered values are p + eps).
  * Label gather via per-example one-hot matmul on the TensorEngine:
    bf16 pipeline: gpsimd cast-DMA (f32->bf16 inline), XBAR dma transpose
    (t,c)->(c,t), matmul E'^T . X^T -> PSUM f32 [65, t], DVE escape copy,
    DMA re-layout into batch-partitioned p_store (f32).
  * Serial DP over T: 4 DVE ops/step with guard columns making the s-1/s-2
    shifts plain AP offsets; the skip-transition mask runs on GPSIMD off the
    critical path; rescale every 8 steps, log-corrections collected in a
    strip and reduced once at the end.
"""

import sys
from contextlib import ExitStack

import numpy as np

for _p in ("/opt/trn_rl_repo",):
    if _p not in sys.path:
        sys.path.insert(0, _p)

import concourse.bass as bass
import concourse.tile as tile
from concourse import mybir
from concourse.bass_utils import run_bass_kernel_spmd

# Problem constants (hardcoded per spec nn_CTC_55808805045003)
B, T, C, L = 1024, 256, 128, 64
NCORES = 8
BL = B // NCORES          # 128 examples per core
S = 2 * L + 1             # 129 extended labels
NS = L + 1                # 65 gather columns (64 labels + blank)
EPS = 1e-7
CH = 128                  # time chunk
NCH = T // CH             # 2
RESC = 8                  # rescale period
GRP = 16                  # examples per cast-DMA group

f32 = mybir.dt.float32
bf16 = mybir.dt.bfloat16
f16 = mybir.dt.float16
i32 = mybir.dt.int32

# gather-pipeline storage dtype for probabilities (bf16 or f16).
# f16 has a 10-bit mantissa (4x finer than bf16); p < 6e-5 lands in f16
# subnormals, which numpy handles exactly and HW needs to not flush.
import os as _os

GDT_NAME = _os.environ.get("CTC_GDT", "f16")
GDT = {"bf16": bf16, "f16": f16}[GDT_NAME]
GDT_NP = {"bf16": None, "f16": np.float16}[GDT_NAME]
ADD = mybir.AluOpType.add
MULT = mybir.AluOpType.mult
ISEQ = mybir.AluOpType.is_equal
NEQ = mybir.AluOpType.not_equal
AX_X = mybir.AxisListType.X
AFT = mybir.ActivationFunctionType


def _body(tc, loss_ap, yp, lab_ap, e_ap):
    nc = tc.nc
    with ExitStack() as ctx:
        const = ctx.enter_context(tc.tile_pool(name="const", bufs=1))
        dstage = ctx.enter_context(tc.tile_pool(name="dstage", bufs=1, space="DRAM"))
        xtp = ctx.enter_context(tc.tile_pool(name="xt", bufs=8))
        gps = ctx.enter_context(tc.tile_pool(name="gpsum", bufs=2, space="PSUM"))
        gsb = ctx.enter_context(tc.tile_pool(name="gsb", bufs=2))
        tiny = ctx.enter_context(tc.tile_pool(name="tiny", bufs=6))

        # ---- label-derived constants (host-computed, DMA'd in) ----
        Eall = const.tile([128, BL * NS], GDT)
        nc.sync.dma_start(Eall[:], e_ap[:, :])
        m_odd = const.tile([128, L], f32)
        nc.sync.dma_start(m_odd[:], lab_ap[:, :])

        # per-chunk gathered probs: p_store[ch][b, s*CH + t] = p(b, ch*CH+t, ext65[b,s]) + eps
        p_stores = [
            const.tile([128, NS * CH], f32, name=f"p_store{ch}") for ch in range(NCH)
        ]
        ps3 = [
            p_stores[ch][:].rearrange("p (s t) -> p s t", s=NS) for ch in range(NCH)
        ]

        ystages = [
            dstage.tile([BL, CH, C], GDT, name=f"ystage{ch}") for ch in range(NCH)
        ]

        def gather_chunk(ch):
            t0 = ch * CH
            ystage = ystages[ch]
            for g in range(BL // GRP):
                # f32 -> f16 cast inline in the DMA (SWDGE), DRAM -> DRAM
                nc.gpsimd.dma_start(
                    ystage[g * GRP : (g + 1) * GRP, :, :],
                    yp[g * GRP : (g + 1) * GRP, t0 : t0 + CH, :],
                )
            for g in range(BL // GRP):
                for i in range(GRP):
                    b = g * GRP + i
                    xt = xtp.tile([C, CH], GDT)
                    nc.sync.dma_start_transpose(xt[:], ystage[b, :, :])
                    j = b % 4
                    if j == 0:
                        gp = gps.tile([NS, 4 * CH], f32)
                    nc.tensor.matmul(
                        gp[:, j * CH : (j + 1) * CH],
                        Eall[:, b * NS : (b + 1) * NS],
                        xt[:],
                        start=True,
                        stop=True,
                    )
                    if j == 3:
                        gs = gsb.tile([NS, 4 * CH], f32)
                        nc.vector.tensor_copy(gs[:], gp[:])
                        for k in range(4):
                            bb = b - 3 + k
                            dst = ps3[ch][bb : bb + 1, :, :]
                            nc.sync.dma_start(dst, gs[:, k * CH : (k + 1) * CH])

        for ch in range(NCH):
            gather_chunk(ch)

        # ---- DP state ----
        # alpha cols: 0,1 = zero guards; 2..130 = s=0..128; 131 pad
        alpha = const.tile([128, 132], f32)
        u = const.tile([128, 132], f32)
        v_odd = const.tile([128, 64], f32)
        aM = const.tile([128, 66], f32)  # col 0 guard; 1..64 = masked odd alphas
        strip = const.tile([128, 32], f32)

        nc.vector.memset(alpha[:], 0.0)
        nc.vector.memset(aM[:], 0.0)

        # t = 0 init: alpha[s=0] = p_blank(t=0), alpha[s=1] = p_lab0(t=0)
        nc.vector.tensor_copy(alpha[:, 2:3], ps3[0][:, NS - 1 : NS, 0:1].squeeze(2))
        nc.vector.tensor_copy(alpha[:, 3:4], ps3[0][:, 0:1, 0:1].squeeze(2))
        # aM[1+j'] = alpha_odd[j'] * m_dest[j'], m_dest[j'] = (lab[j'+1] != lab[j'])
        nc.gpsimd.tensor_tensor(aM[:, 1:2], alpha[:, 3:4], m_odd[:, 0:1], MULT)

        k_resc = 0
        for t in range(1, T):
            p3 = ps3[t // CH]
            tt = t % CH
            p_lab = p3[:, 0:64, tt : tt + 1].squeeze(2)
            p_bl = p3[:, 64:65, tt : tt + 1]
            # u[s] = alpha[s] + alpha[s-1]
            nc.vector.tensor_tensor(u[:, 2:131], alpha[:, 2:131], alpha[:, 1:130], ADD)
            # v_odd[j] = u[2j+3] + aM_prev[j-1]
            nc.vector.tensor_tensor(v_odd[:], u[:, 3:130:2], aM[:, 0:64], ADD)
            # alpha_odd = v_odd * p_lab
            nc.vector.tensor_tensor(alpha[:, 3:130:2], v_odd[:], p_lab, MULT)
            # alpha_even = u_even * p_blank
            nc.vector.tensor_scalar(alpha[:, 2:131:2], u[:, 2:131:2], p_bl, None, MULT)
            # masked odd alphas for the next step's skip term (off critical path);
            # source j'=0..62 feeds destination j'+1, gated by m_dest[j']
            nc.gpsimd.tensor_tensor(
                aM[:, 1:64], alpha[:, 3:128:2], m_odd[:, 0:63], MULT
            )
            if t % RESC == 0:
                cs = tiny.tile([128, 1], f32)
                nc.vector.tensor_reduce(cs[:], alpha[:, 2:131], AX_X, ADD)
                r = tiny.tile([128, 1], f32)
                nc.vector.reciprocal(r[:], cs[:])
                nc.vector.tensor_scalar(alpha[:, 2:131], alpha[:, 2:131], r[:], None, MULT)
                nc.gpsimd.tensor_scalar(aM[:, 1:65], aM[:, 1:65], r[:], None, MULT)
                nc.scalar.activation(strip[:, k_resc : k_resc + 1], cs[:], AFT.Ln)
                k_resc += 1

        # loss = -(sum_k log c_k + log(alpha[S-1] + alpha[S-2]))
        lik = tiny.tile([128, 1], f32)
        nc.vector.tensor_tensor(lik[:], alpha[:, 129:130], alpha[:, 130:131], ADD)
        nc.scalar.activation(strip[:, 31:32], lik[:], AFT.Ln)
        assert k_resc == 31
        slog = tiny.tile([128, 1], f32)
        nc.vector.tensor_reduce(slog[:], strip[:], AX_X, ADD)
        lout = tiny.tile([128, 1], f32)
        nc.vector.tensor_scalar(lout[:], slog[:], -1.0, None, MULT)
        nc.sync.dma_start(loss_ap[:, :], lout[:])


def build_nc():
    nc = bass.Bass("TRN2", target_bir_lowering=False, debug=False)
    yp = nc.dram_tensor("y_pred", [BL, T, C], f32, kind="ExternalInput").ap()
    lab = nc.dram_tensor("m_odd", [BL, L], f32, kind="ExternalInput").ap()
    e_in = nc.dram_tensor("e_all", [128, BL * NS], GDT, kind="ExternalInput").ap()
    loss = nc.dram_tensor("loss", [BL, 1], f32, kind="ExternalOutput").ap()
    with tile.TileContext(nc) as tc:
        _body(tc, loss, yp, lab, e_in)
    return nc


def host_label_consts(y_true):
    """E' one-hot (+eps, bf16) and skip-mask, per core: pure functions of labels."""
    import ml_dtypes

    lab = np.asarray(y_true).astype(np.int64)  # [B, L]
    outs = []
    for i in range(NCORES):
        lb = lab[i * BL : (i + 1) * BL]  # [128, 64]
        ext = np.concatenate(
            [lb, np.full((BL, 1), C - 1, np.int64)], axis=1
        )  # [128, 65]
        e = (np.arange(128)[:, None, None] == ext[None, :, :]).astype(np.float32)
        npdt = GDT_NP or ml_dtypes.bfloat16
        e = (e + EPS).astype(npdt).reshape(128, BL * NS)
        # destination-indexed skip mask: m[j'] = (lab[j'+1] != lab[j']), j'=0..62
        m = np.zeros((BL, L), np.float32)
        m[:, 0:63] = (lb[:, 1:] != lb[:, :-1]).astype(np.float32)
        outs.append((e, m))
    return outs


_CACHE = {}

# --- BIR legalizer -----------------------------------------------------------
# This container's walrus encodes at most ONE sync wait on SP-queue
# instruction classes (PSEUDO_DMA_DIRECT2D / XPOSE / CTRL): "Too many sync
# wait commands". Tile freely emits >=2 waits per instruction. Split the
# extras onto NoOps inserted just before (same engine stream => semantics
# preserved, waits satisfied in order).
_SPLIT_OPS = {"DMACopy", "DmaTransposeAnt", "DMAGatherAnt", "Drain", "NoOp"}


def _legalize_bir(bir_bytes):
    import orjson

    d = orjson.loads(bir_bytes)
    n_new = 0
    for fn in d.get("functions", []):
        for blk in fn.get("blocks", []):
            insts = blk.get("instructions")
            if not insts:
                continue
            out = []
            for ins in insts:
                si = ins.get("sync_info")
                if si:
                    waits = si.get("on_wait") or []
                    if len(waits) > 1:
                        for w in waits[:-1]:
                            n_new += 1
                            out.append(
                                {
                                    "debug": ins.get("debug", 0),
                                    "engine": ins["engine"],
                                    "ins": [],
                                    "outs": [],
                                    "name": f"ZW-{n_new}",
                                    "opcode": "NoOp",
                                    "sync_info": {"on_wait": [w], "on_update": []},
                                }
                            )
                        si["on_wait"] = [waits[-1]]
                out.append(ins)
            blk["instructions"] = out
    return orjson.dumps(d)


def _install_bir_legalizer():
    import concourse.bass2jax as b2j

    if getattr(b2j, "_ctc_legalizer_installed", False):
        return
    orig = b2j.compile_bir_kernel

    def wrapper(bir_json, tmpdir, neff_name="file.neff"):
        bir_json = _legalize_bir(bir_json)
        return orig(bir_json, tmpdir, neff_name=neff_name)

    b2j.compile_bir_kernel = wrapper
    b2j._ctc_legalizer_installed = True


def kernel(y_true, y_pred):
    assert y_pred.shape == (B, T, C) and y_true.shape == (B, L)
    _install_bir_legalizer()
    nc = _CACHE.get("nc")
    if nc is None:
        nc = _CACHE["nc"] = build_nc()
    yp = np.ascontiguousarray(y_pred, dtype=np.float32)
    consts = host_label_consts(y_true)
    in_maps = [
        {
            "y_pred": yp[i * BL : (i + 1) * BL],
            "m_odd": consts[i][1],
            "e_all": consts[i][0],
        }
        for i in range(NCORES)
    ]
    res = run_bass_kernel_spmd(nc, in_maps, list(range(NCORES)))
    out = np.concatenate([res.results[i]["loss"] for i in range(NCORES)], axis=0)
    return out.astype(np.float32)
